# revision 3
# baseline (speedup 1.0000x reference)
"""DocGCN Trainium2 kernel v2: fp8 DoubleRow matmuls + log-mask-in-PE attention.

Per core = one doc (1024 nodes, block-diagonal graph). Layout tricks:
  - h-dims permuted s.t. sigma-chunk mi=(2m'+i) holds dims 256m'+2p+i: makes
    hT directly usable as DoubleRow (K=256) lhsT for z = h @ W.
  - P tiles [128, 2, 512] fp8e5 (sub-row i = node chunk 2c'+i) feed DoubleRow
    agg; global 2^-8 scale folded into Exp bias (cancels in normalization).
  - Edge mask folded as lnM (-200 non-edge) into ES psum via identity matmul.
  - el/er come free as extra z columns; one PE transpose + SBUF DMA makes rows.
  - elu(y) = min(exp(y)-1, relu(y)); leaky-relu via STT max(x, 0.2x).
"""

import numpy as np

SPD = 8          # sentences per doc
L = 512          # tokens per sentence
H = 768          # hidden
SEL = 128        # selected spans (graph nodes) per sentence
NPD = SPD * SEL  # nodes per doc = 1024
NCH = NPD // 128  # 8 node chunks
NSC = NCH // 2    # 4 node super-chunks (DoubleRow)
KH2 = 3          # h super-chunks of 256
SW = 144         # smask width: 128 sel cols + qmask col + pad (16B align)
WC = 784         # W' cols: 768 + el + er + pad (16B align)
D = 8            # docs = cores
NEG = 0.2
PSC = -8.0 * float(np.log(2.0))   # exp bias: global 2^-8 attention scale

_PROG = {}


def _ensure_env():
    import sys, types
    for p in ("/opt/trn_rl_repo", "/opt/trn_rl_repo/concourse"):
        if p not in sys.path:
            sys.path.insert(0, p)
    if "antenv.axon_hooks" not in sys.modules:
        try:
            import antenv
            mod = types.ModuleType("antenv.axon_hooks")
            mod._hook = None
            mod.set_axon_ntff_profile_hook = lambda h: setattr(mod, "_hook", h)
            mod.get_axon_ntff_profile_hook = lambda: mod._hook
            sys.modules["antenv.axon_hooks"] = mod
            antenv.axon_hooks = mod
            if "/root/.axon_site" not in sys.path:
                sys.path.insert(0, "/root/.axon_site")
            from trn_agent_boot import trn_boot
            h = trn_boot._ntff_profile_via_ctypes("/opt/axon/libaxon_pjrt.so")
            if h is not None:
                mod.set_axon_ntff_profile_hook(h)
        except Exception:
            pass


def _perm():
    """sigma: permuted position (2m'+i)*128+p  <-  original dim 256m'+2p+i."""
    perm = np.empty(H, np.int64)
    for mp in range(KH2):
        for i in range(2):
            for p in range(128):
                perm[(2 * mp + i) * 128 + p] = 256 * mp + 2 * p + i
    return perm


def _build_program():
    import concourse.bacc as bacc
    import concourse.tile as tile
    from concourse import mybir
    from contextlib import ExitStack

    f32 = mybir.dt.float32
    bf16 = mybir.dt.bfloat16
    f8e4 = mybir.dt.float8e4
    f8e5 = mybir.dt.float8e5
    AF = mybir.ActivationFunctionType
    OP = mybir.AluOpType
    AX = mybir.AxisListType
    DR = mybir.MatmulPerfMode.DoubleRow

    nc = bacc.Bacc(None, target_bir_lowering=False)

    fsd = nc.dram_tensor("fs", [SPD, 128, 2, 2, H + SW], f8e4,
                         kind="ExternalInput")
    lmd = nc.dram_tensor("lm", [NCH, 128, NPD], f8e4, kind="ExternalInput")
    wtd = [nc.dram_tensor(f"W{i}", [128, KH2, 2, WC], f8e4, kind="ExternalInput")
           for i in range(3)]
    idb = nc.dram_tensor("idb", [128, 128], f32, kind="ExternalInput")
    idf = nc.dram_tensor("idf", [128, 128], f8e4, kind="ExternalInput")
    out_d = nc.dram_tensor("out", [1, 1], f32, kind="ExternalOutput")
    oneer_d = nc.dram_tensor("oneer_d", [2, NPD], bf16, kind="Internal")
    zo_d = nc.dram_tensor("zo", [2, NPD], bf16, kind="ExternalInput")

    with tile.TileContext(nc) as tc:
        with ExitStack() as ctx:
            const = ctx.enter_context(tc.tile_pool(name="const", bufs=1))
            fpool = ctx.enter_context(tc.tile_pool(name="fpool", bufs=2))
            spool = ctx.enter_context(tc.tile_pool(name="spool", bufs=2))
            tpool = ctx.enter_context(tc.tile_pool(name="tpool", bufs=6))
            ppool = ctx.enter_context(tc.tile_pool(name="ppool", bufs=8))
            ypool = ctx.enter_context(tc.tile_pool(name="ypool", bufs=4))
            epool = ctx.enter_context(tc.tile_pool(name="epool", bufs=4))
            rpool = ctx.enter_context(tc.tile_pool(name="rpool", bufs=4))
            rbp = ctx.enter_context(tc.tile_pool(name="rbp", bufs=2))
            ps = ctx.enter_context(tc.tile_pool(name="ps", bufs=8, space="PSUM"))

            # ---- constants / persistent tiles ----
            identb = const.tile([128, 128], f32, name="identb", tag="identb")
            nc.sync.dma_start(out=identb[:], in_=idb[:])
            identf = const.tile([128, 128], f8e4, name="identf", tag="identf")
            nc.sync.dma_start(out=identf[:], in_=idf[:])
            wt = []
            for i in range(3):
                w = const.tile([128, KH2, 2, WC], f8e4, name=f"wt{i}", tag=f"wt{i}")
                wt.append(w)
            ones_dr = const.tile([128, 2, 128], f8e4, name="ones_dr", tag="ones_dr")
            nc.vector.memset(ones_dr[:], 1.0)
            onesrow = const.tile([1, NPD], bf16, name="onesrow", tag="onesrow")
            nc.vector.memset(onesrow[:], 1.0)
            # seed the static ones row of the DRAM bounce buffer
            nc.sync.dma_start(out=oneer_d[0:1, :], in_=onesrow[:])
            oneer = const.tile([2, NPD], bf16, name="oneer", tag="oneer")
            oneer2 = const.tile([2, NPD], bf16, name="oneer2", tag="oneer2")
            nc.sync.dma_start(out=oneer2[:], in_=zo_d[:])
            hta = const.tile([128, KH2, 2, NPD], f8e4, name="hta", tag="hta")
            htb = const.tile([128, KH2, 2, NPD], f8e4, name="htb", tag="htb")
            zt = const.tile([128, NCH, WC], f8e4, name="zt", tag="zt")
            h3s = [const.tile([128, 512], bf16, name=f"h3s{m}", tag=f"h3s{m}")
                   for m in range(2)]
            qfacc = const.tile([128, 6, SPD], f32, name="qfacc", tag="qfacc")
            avacc = const.tile([128, 12], f32, name="avacc", tag="avacc")
            elerc = const.tile([128, 16], f32, name="elerc", tag="elerc")
            elpa = const.tile([128, 8], f32, name="elpa", tag="elpa")
            elpb = const.tile([128, 8], f32, name="elpb", tag="elpb")
            errow8 = const.tile([8, 128], bf16, name="errow8", tag="errow8")
            dfin = const.tile([128, 6], f32, name="dfin", tag="dfin")
            pscb = const.tile([128, 1], f32, name="pscb", tag="pscb")
            nc.vector.memset(pscb[:], PSC)
            zeros = const.tile([128, 512], bf16, name="zeros", tag="zeros")
            nc.vector.memset(zeros[:], 0.0)
            lm = const.tile([128, NCH, NPD], f8e4, name="lm", tag="lm")

            # ---------------- helpers ----------------
            hts = [hta, htb]

            def z_chunk(li, c):
                """z = h @ W' for node chunk c (DoubleRow fp8)."""
                hin = hts[li % 2]
                W = wt[li]
                pA = ps.tile([128, 512], f32, name="ps", tag="ps")
                pB = ps.tile([128, 512], f32, name="ps", tag="ps")
                for mp in range(KH2):
                    lhsT = hin[:, mp, :, c * 128:(c + 1) * 128]
                    nc.tensor.matmul(pA[:], lhsT, W[:, mp, :, 0:512],
                                     start=(mp == 0), stop=(mp == KH2 - 1),
                                     perf_mode=DR)
                    nc.tensor.matmul(pB[:, 0:272], lhsT, W[:, mp, :, 512:WC],
                                     start=(mp == 0), stop=(mp == KH2 - 1),
                                     perf_mode=DR)
                nc.scalar.copy(zt[:, c, 0:512], pA[:])
                nc.vector.tensor_copy(zt[:, c, 512:770], pB[:, 0:258])
                nc.vector.tensor_copy(elerc[:, 2 * c:2 * c + 2],
                                      pB[:, 256:258])

            def elrow_prep():
                """er cols -> row via PE transpose + DRAM bounce; el -> exp
                bias tiles (el enters the logits as a per-partition bias)."""
                nc.vector.tensor_scalar(elpa[:], elerc[:, 0:16:2], 1.0, PSC,
                                        OP.mult, OP.add)
                nc.vector.tensor_scalar(elpb[:], elerc[:, 0:16:2], NEG, PSC,
                                        OP.mult, OP.add)
                tpB = ps.tile([128, 512], f32, name="ps", tag="ps")
                nc.tensor.transpose(tpB[0:8, 0:128], elerc[:, 1:16:2], identb[:])
                nc.vector.tensor_copy(errow8[:], tpB[0:8, 0:128])
                nc.sync.dma_start(out=oneer_d[1:2, :], in_=errow8[:])
                nc.sync.dma_start(out=oneer[:], in_=oneer_d[:])

            # ---------------- span extraction + layer-0 z ----------------
            fstiles = {}

            def fs_dma(s):
                fs = fpool.tile([128, 2, 2, H + SW], f8e4, name="fs", tag="fs")
                nc.sync.dma_start(out=fs[:], in_=fsd[s])
                fstiles[s] = fs

            fs_dma(0)
            fs_dma(1)
            for s in range(SPD):
                fs = fstiles[s]
                if s + 2 < SPD:
                    fs_dma(s + 2)
                for mi in range(6):
                    p = ps.tile([128, 512], f32, name="ps", tag="ps")
                    for k2 in range(2):
                        for i2 in range(2):
                            nc.tensor.matmul(
                                p[:, 0:129],
                                fs[:, k2, i2, mi * 128:(mi + 1) * 128],
                                fs[:, k2, i2, H:H + 129],
                                start=(k2 == 0 and i2 == 0),
                                stop=(k2 == 1 and i2 == 1))
                    if (s * 6 + mi) % 2 == 0:
                        nc.scalar.copy(
                            hta[:, mi // 2, mi % 2, s * 128:(s + 1) * 128],
                            p[:, 0:128])
                    else:
                        nc.vector.tensor_copy(
                            hta[:, mi // 2, mi % 2, s * 128:(s + 1) * 128],
                            p[:, 0:128])
                    nc.vector.tensor_copy(qfacc[:, mi, s:s + 1], p[:, 128:129])
                # stagger big constant loads behind the first feature tiles
                if s == 0:
                    nc.sync.dma_start(out=wt[0][:], in_=wtd[0][:])
                elif s == 2:
                    for c in range(4):
                        nc.sync.dma_start(out=lm[:, c, :], in_=lmd[c])
                elif s == 3:
                    for c in range(4, NCH):
                        nc.sync.dma_start(out=lm[:, c, :], in_=lmd[c])
                elif s == 4:
                    nc.sync.dma_start(out=wt[1][:], in_=wtd[1][:])
                elif s == 5:
                    nc.sync.dma_start(out=wt[2][:], in_=wtd[2][:])
                # layer-0 z for node chunk s (nodes of sentence s)
                z_chunk(0, s)

            # ---------------- GAT layers ----------------
            for li in range(3):
                hout = hts[(li + 1) % 2]
                elrow_prep()
                P = {}
                esq = {}

                def es_lm(half, c):
                    es = ps.tile([128, 512], f32, name="ps", tag="ps")
                    nc.tensor.matmul(es[:], identf[:],
                                     lm[:, c, half * 512:half * 512 + 512],
                                     start=True, stop=False)
                    esq[(half, c)] = es

                def es_fin(half, c):
                    es = esq[(half, c)]
                    nc.tensor.matmul(es[:], oneer2[:, c * 128:(c + 1) * 128],
                                     oneer[:, half * 512:half * 512 + 512],
                                     start=False, stop=True)
                    a = tpool.tile([128, 512], bf16, name="a", tag="a")
                    nc.scalar.activation(a[:], es[:], AF.Exp,
                                         bias=elpa[:, c:c + 1])
                    b = tpool.tile([128, 512], bf16, name="b", tag="b")
                    nc.scalar.activation(b[:], es[:], AF.Exp,
                                         bias=elpb[:, c:c + 1], scale=NEG)
                    if c % 2 == 0:
                        P[(half, c // 2)] = ppool.tile(
                            [128, 2, 512], f8e5, name="P", tag="P")
                    nc.vector.tensor_max(P[(half, c // 2)][:, c % 2, :],
                                         a[:], b[:])

                def csum(half, cp, csp):
                    nc.tensor.matmul(csp[:], ones_dr[:], P[(half, cp)][:],
                                     start=(cp == 0), stop=(cp == NSC - 1),
                                     perf_mode=DR)

                def recip(csp):
                    rb = rbp.tile([128, 512], f32, name="rb", tag="rb")
                    nc.vector.reciprocal_approx_fast(rb[:], csp[:])
                    return rb

                def agg_mm(half, cp, aggp):
                    for mi in range(6):
                        nc.tensor.matmul(
                            aggp[mi][:],
                            zt[:, 2 * cp:2 * cp + 2, mi * 128:(mi + 1) * 128],
                            P[(half, cp)][:],
                            start=(cp == 0), stop=(cp == NSC - 1),
                            perf_mode=DR)

                def yelu_mi(half, rb, aggt, mi):
                    y = ypool.tile([128, 512], bf16, name="y", tag="y")
                    nc.vector.tensor_mul(y[:], aggt[:], rb[:])
                    e = epool.tile([128, 512], bf16, name="e", tag="e")
                    nc.scalar.activation(e[:], y[:], AF.Exp)
                    r = rpool.tile([128, 512], bf16, name="r", tag="r")
                    if li == 2 or mi % 2 == 0:
                        nc.scalar.activation(r[:], y[:], AF.Relu)
                    else:
                        nc.vector.tensor_scalar_max(r[:], y[:], 0.0)
                    # elu = min(exp(y)-1, relu(y))
                    if li == 2:
                        dst = h3s[mi % 2][:]
                        nc.vector.scalar_tensor_tensor(
                            dst, e[:], 1.0, r[:], OP.subtract, OP.min,
                            accum_out=avacc[:, half * 6 + mi:half * 6 + mi + 1])
                    else:
                        dst = hout[:, mi // 2, mi % 2,
                                   half * 512:half * 512 + 512]
                        nc.vector.scalar_tensor_tensor(
                            dst, e[:], 1.0, r[:], OP.subtract, OP.min)

                def agg4(half, aggt, mi):
                    for cp in range(NSC):
                        nc.tensor.matmul(
                            aggt[:],
                            zt[:, 2 * cp:2 * cp + 2, mi * 128:(mi + 1) * 128],
                            P[(half, cp)][:],
                            start=(cp == 0), stop=(cp == NSC - 1),
                            perf_mode=DR)

                # half-0 P production (LM adds first to cover el/er DMA chain)
                for c in range(NCH):
                    es_lm(0, c)
                for c in range(NCH):
                    es_fin(0, c)
                csp0 = ps.tile([128, 512], f32, name="ps", tag="ps")
                for cp in range(NSC):
                    csum(0, cp, csp0)
                rb0 = recip(csp0)
                # agg half-0 (cp-outer: banks rotate, PE pipelines)
                # overlapped with half-1 P production (es double-buffered)
                aggp0 = [ps.tile([128, 512], f32, name="ps", tag="ps")
                         for _ in range(6)]
                for cp in range(NSC):
                    for mi in range(6):
                        nc.tensor.matmul(
                            aggp0[mi][:],
                            zt[:, 2 * cp:2 * cp + 2, mi * 128:(mi + 1) * 128],
                            P[(0, cp)][:],
                            start=(cp == 0), stop=(cp == NSC - 1),
                            perf_mode=DR)
                    es_lm(1, 2 * cp)
                    es_fin(1, 2 * cp)
                    es_lm(1, 2 * cp + 1)
                    es_fin(1, 2 * cp + 1)
                csp1 = ps.tile([128, 512], f32, name="ps", tag="ps")
                for cp in range(NSC):
                    csum(1, cp, csp1)
                rb1 = recip(csp1)
                for mi in range(6):
                    yelu_mi(0, rb0, aggp0[mi], mi)
                # agg half-1 overlapped with next layer's first z chunks
                aggp1 = [ps.tile([128, 512], f32, name="ps", tag="ps")
                         for _ in range(6)]
                for cp in range(NSC):
                    for mi in range(6):
                        nc.tensor.matmul(
                            aggp1[mi][:],
                            zt[:, 2 * cp:2 * cp + 2, mi * 128:(mi + 1) * 128],
                            P[(1, cp)][:],
                            start=(cp == 0), stop=(cp == NSC - 1),
                            perf_mode=DR)
                    if li < 2 and cp >= 1:
                        z_chunk(li + 1, cp - 1)
                if li < 2:
                    z_chunk(li + 1, 3)
                for mi in range(6):
                    yelu_mi(1, rb1, aggp1[mi], mi)
                if li < 2:
                    for c in range(4, NCH):
                        z_chunk(li + 1, c)

            # ---------------- final reduction (batched) ----------------
            qfT6 = rpool.tile([128, 6], f32, name="qfT6", tag="qfT6")
            nc.vector.tensor_reduce(qfT6[:], qfacc[:], AX.X, OP.add)
            u6 = rpool.tile([128, 6], f32, name="u6", tag="u6")
            nc.vector.tensor_add(u6[:], avacc[:, 0:6], avacc[:, 6:12])
            nc.vector.scalar_tensor_tensor(
                dfin[:], u6[:], 1.0 / NPD, qfT6[:], OP.mult, OP.subtract)
            dfa = rpool.tile([128, 6], f32, name="dfa", tag="dfa")
            nc.scalar.activation(dfa[:], dfin[:], AF.Abs)
            dfr = rpool.tile([128, 1], f32, name="dfr", tag="dfr")
            nc.vector.tensor_reduce(dfr[:], dfa[:], AX.X, OP.add)
            from concourse import bass_isa
            finp = rpool.tile([128, 1], f32, name="finp", tag="finp")
            nc.gpsimd.partition_all_reduce(finp[:], dfr[:], 128,
                                           bass_isa.ReduceOp.add)
            nc.sync.dma_start(out=out_d[:], in_=finp[0:1, :])

    nc.finalize()
    return nc


def _shard_inputs(inputs):
    """Host-side preprocessing: build per-core input maps."""
    import ml_dtypes
    f8 = ml_dtypes.float8_e4m3
    perm = _perm()

    f = np.asarray(inputs["features"], np.float32)
    spans = np.asarray(inputs["token_spans"])
    masks = np.asarray(inputs["masks"])
    sel = np.asarray(inputs["selected_indices"])
    src = np.asarray(inputs["src"])
    dst = np.asarray(inputs["dst"])
    doc_spans = np.asarray(inputs["doc_spans"])
    seg = np.asarray(inputs["segment_ids"])
    ish = np.asarray(inputs["is_head"])
    B = f.shape[0]

    pos = np.arange(L)
    bi = np.arange(B)[:, None]
    stx = spans[bi, sel, 0]
    en = spans[bi, sel, 1]
    sm = ((pos[None, None, :] >= stx[:, :, None])
          & (pos[None, None, :] < en[:, :, None])
          & (masks[:, None, :] > 0)).astype(np.float32)
    cnt = np.maximum(sm.sum(-1), 1.0)
    w = sm / cnt[:, :, None] * (en > 0).astype(np.float32)[:, :, None]  # [B,SEL,L]
    doc_cnt = np.maximum((doc_spans[:, 1] - doc_spans[:, 0]).astype(np.float32), 1.0)
    qm = (((ish != 2) & (seg == 0) & (masks > 0)).astype(np.float32))   # [B,L]
    smask_all = np.zeros((B, L, SW), np.float32)
    smask_all[:, :, :SEL] = w.transpose(0, 2, 1)

    # shared weight tensors
    wts = []
    for i in range(3):
        W = np.asarray(inputs[f"W{i}"], np.float32)
        al = np.asarray(inputs[f"al{i}"], np.float32)
        ar = np.asarray(inputs[f"ar{i}"], np.float32)
        Wp = W[perm][:, perm]
        wal = (W @ al)[perm]
        war = (W @ ar)[perm]
        Wf = np.zeros((H, WC), np.float32)
        Wf[:, :H] = Wp
        Wf[:, H] = wal
        Wf[:, H + 1] = war
        wts.append(Wf.reshape(KH2, 2, 128, WC).transpose(2, 0, 1, 3).astype(f8))
    identb = np.eye(128, dtype=np.float32)
    identf = np.eye(128, dtype=f8)

    in_maps = []
    for d in range(D):
        s0, s1 = int(doc_spans[d, 0]), int(doc_spans[d, 1])
        assert s1 - s0 == SPD, "kernel assumes 8 sentences per doc"
        sm_d = smask_all[s0:s1].copy()
        sm_d[:, :, SEL] = qm[s0:s1] / doc_cnt[d]
        f_d = f[s0:s1][:, :, perm]                      # [SPD, L, H] permuted
        lo, hi = d * NPD, (d + 1) * NPD
        eidx = np.where((dst >= lo) & (dst < hi))[0]
        ls = src[eidx] - lo
        ld = dst[eidx] - lo
        assert np.all((ls >= 0) & (ls < NPD)), "edge crosses doc block"
        M = np.bincount(ls * NPD + ld, minlength=NPD * NPD).astype(np.float32)
        M = M.reshape(NPD, NPD)
        LM = np.where(M > 0, np.log(np.maximum(M, 1.0)), -200.0).astype(np.float32)
        fs = np.concatenate([f_d, sm_d], axis=2)     # [SPD, L, H+SW]
        im = {
            # L interleave: l = 256*k2 + 2p + i
            "fs": fs.reshape(SPD, 2, 128, 2, H + SW).transpose(0, 2, 1, 3, 4)
                    .astype(f8),
            "lm": LM.reshape(NCH, 128, NPD).astype(f8),
            "idb": identb, "idf": identf,
            "zo": np.concatenate([np.zeros((1, NPD), np.float32),
                                  np.ones((1, NPD), np.float32)]
                                 ).astype(ml_dtypes.bfloat16),
        }
        for i in range(3):
            im[f"W{i}"] = wts[i]
        in_maps.append(im)
    return in_maps


def _run(inputs, trace=False, tmpdir=None):
    _ensure_env()
    from concourse.bass_utils import run_bass_kernel_spmd
    if "nc" not in _PROG:
        _PROG["nc"] = _build_program()
    in_maps = _shard_inputs(inputs)
    res = run_bass_kernel_spmd(_PROG["nc"], in_maps, core_ids=list(range(D)),
                               trace=trace, tmpdir=tmpdir)
    out = np.array([res.results[c]["out"][0, 0] for c in range(D)], np.float32)
    return out, res


def kernel(**inputs) -> np.ndarray:
    out, _ = _run(inputs)
    return out


# revision 4
# speedup vs baseline: 1.0006x; 1.0006x over previous
"""DocGCN Trainium2 kernel v2: fp8 DoubleRow matmuls + log-mask-in-PE attention.

Per core = one doc (1024 nodes, block-diagonal graph). Layout tricks:
  - h-dims permuted s.t. sigma-chunk mi=(2m'+i) holds dims 256m'+2p+i: makes
    hT directly usable as DoubleRow (K=256) lhsT for z = h @ W.
  - P tiles [128, 2, 512] fp8e5 (sub-row i = node chunk 2c'+i) feed DoubleRow
    agg; global 2^-8 scale folded into Exp bias (cancels in normalization).
  - Edge mask folded as lnM (-200 non-edge) into ES psum via identity matmul.
  - el/er come free as extra z columns; one PE transpose + SBUF DMA makes rows.
  - elu(y) = min(exp(y)-1, relu(y)); leaky-relu via STT max(x, 0.2x).
"""

import numpy as np

SPD = 8          # sentences per doc
L = 512          # tokens per sentence
H = 768          # hidden
SEL = 128        # selected spans (graph nodes) per sentence
NPD = SPD * SEL  # nodes per doc = 1024
NCH = NPD // 128  # 8 node chunks
NSC = NCH // 2    # 4 node super-chunks (DoubleRow)
KH2 = 3          # h super-chunks of 256
SW = 144         # smask width: 128 sel cols + qmask col + pad (16B align)
WC = 784         # W' cols: 768 + el + er + pad (16B align)
D = 8            # docs = cores
NEG = 0.2
PSC = -8.0 * float(np.log(2.0))   # exp bias: global 2^-8 attention scale

_PROG = {}


def _ensure_env():
    import sys, types
    for p in ("/opt/trn_rl_repo", "/opt/trn_rl_repo/concourse"):
        if p not in sys.path:
            sys.path.insert(0, p)
    if "antenv.axon_hooks" not in sys.modules:
        try:
            import antenv
            mod = types.ModuleType("antenv.axon_hooks")
            mod._hook = None
            mod.set_axon_ntff_profile_hook = lambda h: setattr(mod, "_hook", h)
            mod.get_axon_ntff_profile_hook = lambda: mod._hook
            sys.modules["antenv.axon_hooks"] = mod
            antenv.axon_hooks = mod
            if "/root/.axon_site" not in sys.path:
                sys.path.insert(0, "/root/.axon_site")
            from trn_agent_boot import trn_boot
            h = trn_boot._ntff_profile_via_ctypes("/opt/axon/libaxon_pjrt.so")
            if h is not None:
                mod.set_axon_ntff_profile_hook(h)
        except Exception:
            pass


def _perm():
    """sigma: permuted position (2m'+i)*128+p  <-  original dim 256m'+2p+i."""
    perm = np.empty(H, np.int64)
    for mp in range(KH2):
        for i in range(2):
            for p in range(128):
                perm[(2 * mp + i) * 128 + p] = 256 * mp + 2 * p + i
    return perm


def _build_program():
    import concourse.bacc as bacc
    import concourse.tile as tile
    from concourse import mybir
    from contextlib import ExitStack

    f32 = mybir.dt.float32
    bf16 = mybir.dt.bfloat16
    f8e4 = mybir.dt.float8e4
    f8e5 = mybir.dt.float8e5
    AF = mybir.ActivationFunctionType
    OP = mybir.AluOpType
    AX = mybir.AxisListType
    DR = mybir.MatmulPerfMode.DoubleRow

    nc = bacc.Bacc(None, target_bir_lowering=False)

    fsd = nc.dram_tensor("fs", [SPD, 128, 2, 2, H + SW], f8e4,
                         kind="ExternalInput")
    lmd = nc.dram_tensor("lm", [NCH, 128, NPD], f8e4, kind="ExternalInput")
    wtd = [nc.dram_tensor(f"W{i}", [128, KH2, 2, WC], f8e4, kind="ExternalInput")
           for i in range(3)]
    idb = nc.dram_tensor("idb", [128, 128], f32, kind="ExternalInput")
    idf = nc.dram_tensor("idf", [128, 128], f8e4, kind="ExternalInput")
    out_d = nc.dram_tensor("out", [1, 1], f32, kind="ExternalOutput")
    oneer_d = nc.dram_tensor("oneer_d", [2, NPD], bf16, kind="Internal")
    zo_d = nc.dram_tensor("zo", [2, NPD], bf16, kind="ExternalInput")

    with tile.TileContext(nc) as tc:
        with ExitStack() as ctx:
            const = ctx.enter_context(tc.tile_pool(name="const", bufs=1))
            fpool = ctx.enter_context(tc.tile_pool(name="fpool", bufs=2))
            spool = ctx.enter_context(tc.tile_pool(name="spool", bufs=2))
            tpool = ctx.enter_context(tc.tile_pool(name="tpool", bufs=6))
            ppool = ctx.enter_context(tc.tile_pool(name="ppool", bufs=8))
            ypool = ctx.enter_context(tc.tile_pool(name="ypool", bufs=4))
            epool = ctx.enter_context(tc.tile_pool(name="epool", bufs=4))
            rpool = ctx.enter_context(tc.tile_pool(name="rpool", bufs=4))
            rbp = ctx.enter_context(tc.tile_pool(name="rbp", bufs=2))
            ps = ctx.enter_context(tc.tile_pool(name="ps", bufs=8, space="PSUM"))

            # ---- constants / persistent tiles ----
            identb = const.tile([128, 128], f32, name="identb", tag="identb")
            identf = const.tile([128, 128], f8e4, name="identf", tag="identf")
            wt = []
            for i in range(3):
                w = const.tile([128, KH2, 2, WC], f8e4, name=f"wt{i}", tag=f"wt{i}")
                wt.append(w)
            ones_dr = const.tile([128, 2, 128], f8e4, name="ones_dr", tag="ones_dr")
            nc.vector.memset(ones_dr[:], 1.0)
            onesrow = const.tile([1, NPD], bf16, name="onesrow", tag="onesrow")
            nc.vector.memset(onesrow[:], 1.0)

            oneer = const.tile([2, NPD], bf16, name="oneer", tag="oneer")
            oneer2 = const.tile([2, NPD], bf16, name="oneer2", tag="oneer2")
            hta = const.tile([128, KH2, 2, NPD], f8e4, name="hta", tag="hta")
            htb = const.tile([128, KH2, 2, NPD], f8e4, name="htb", tag="htb")
            zt = const.tile([128, NCH, WC], f8e4, name="zt", tag="zt")
            h3s = [const.tile([128, 512], bf16, name=f"h3s{m}", tag=f"h3s{m}")
                   for m in range(2)]
            qfacc = const.tile([128, 6, SPD], f32, name="qfacc", tag="qfacc")
            avacc = const.tile([128, 12], f32, name="avacc", tag="avacc")
            elerc = const.tile([128, 16], f32, name="elerc", tag="elerc")
            elpa = const.tile([128, 8], f32, name="elpa", tag="elpa")
            elpb = const.tile([128, 8], f32, name="elpb", tag="elpb")
            errow8 = const.tile([8, 128], bf16, name="errow8", tag="errow8")
            dfin = const.tile([128, 6], f32, name="dfin", tag="dfin")
            pscb = const.tile([128, 1], f32, name="pscb", tag="pscb")
            nc.vector.memset(pscb[:], PSC)
            zeros = const.tile([128, 512], bf16, name="zeros", tag="zeros")
            nc.vector.memset(zeros[:], 0.0)
            lm = const.tile([128, NCH, NPD], f8e4, name="lm", tag="lm")

            # ---------------- helpers ----------------
            hts = [hta, htb]

            def z_chunk(li, c):
                """z = h @ W' for node chunk c (DoubleRow fp8)."""
                hin = hts[li % 2]
                W = wt[li]
                pA = ps.tile([128, 512], f32, name="ps", tag="ps")
                pB = ps.tile([128, 512], f32, name="ps", tag="ps")
                for mp in range(KH2):
                    lhsT = hin[:, mp, :, c * 128:(c + 1) * 128]
                    nc.tensor.matmul(pA[:], lhsT, W[:, mp, :, 0:512],
                                     start=(mp == 0), stop=(mp == KH2 - 1),
                                     perf_mode=DR)
                    nc.tensor.matmul(pB[:, 0:272], lhsT, W[:, mp, :, 512:WC],
                                     start=(mp == 0), stop=(mp == KH2 - 1),
                                     perf_mode=DR)
                nc.scalar.copy(zt[:, c, 0:512], pA[:])
                nc.vector.tensor_copy(zt[:, c, 512:770], pB[:, 0:258])
                nc.vector.tensor_copy(elerc[:, 2 * c:2 * c + 2],
                                      pB[:, 256:258])

            def elrow_prep():
                """er cols -> row via PE transpose + DRAM bounce; el -> exp
                bias tiles (el enters the logits as a per-partition bias)."""
                nc.vector.tensor_scalar(elpa[:], elerc[:, 0:16:2], 1.0, PSC,
                                        OP.mult, OP.add)
                nc.vector.tensor_scalar(elpb[:], elerc[:, 0:16:2], NEG, PSC,
                                        OP.mult, OP.add)
                tpB = ps.tile([128, 512], f32, name="ps", tag="ps")
                nc.tensor.transpose(tpB[0:8, 0:128], elerc[:, 1:16:2], identb[:])
                nc.vector.tensor_copy(errow8[:], tpB[0:8, 0:128])
                nc.sync.dma_start(out=oneer_d[1:2, :], in_=errow8[:])
                nc.sync.dma_start(out=oneer[:], in_=oneer_d[:])

            # ---------------- span extraction + layer-0 z ----------------
            fstiles = {}

            def fs_dma(s):
                fs = fpool.tile([128, 2, 2, H + SW], f8e4, name="fs", tag="fs")
                nc.sync.dma_start(out=fs[:], in_=fsd[s])
                fstiles[s] = fs

            fs_dma(0)
            fs_dma(1)
            nc.sync.dma_start(out=identf[:], in_=idf[:])
            nc.sync.dma_start(out=identb[:], in_=idb[:])
            nc.sync.dma_start(out=oneer2[:], in_=zo_d[:])
            nc.sync.dma_start(out=oneer_d[0:1, :], in_=onesrow[:])
            for s in range(SPD):
                fs = fstiles[s]
                if s + 2 < SPD:
                    fs_dma(s + 2)
                for mi in range(6):
                    p = ps.tile([128, 512], f32, name="ps", tag="ps")
                    for k2 in range(2):
                        for i2 in range(2):
                            nc.tensor.matmul(
                                p[:, 0:129],
                                fs[:, k2, i2, mi * 128:(mi + 1) * 128],
                                fs[:, k2, i2, H:H + 129],
                                start=(k2 == 0 and i2 == 0),
                                stop=(k2 == 1 and i2 == 1))
                    if (s * 6 + mi) % 2 == 0:
                        nc.scalar.copy(
                            hta[:, mi // 2, mi % 2, s * 128:(s + 1) * 128],
                            p[:, 0:128])
                    else:
                        nc.vector.tensor_copy(
                            hta[:, mi // 2, mi % 2, s * 128:(s + 1) * 128],
                            p[:, 0:128])
                    nc.vector.tensor_copy(qfacc[:, mi, s:s + 1], p[:, 128:129])
                # stagger big constant loads behind the first feature tiles
                if s == 0:
                    nc.sync.dma_start(out=wt[0][:], in_=wtd[0][:])
                elif s == 2:
                    for c in range(4):
                        nc.sync.dma_start(out=lm[:, c, :], in_=lmd[c])
                elif s == 3:
                    for c in range(4, NCH):
                        nc.sync.dma_start(out=lm[:, c, :], in_=lmd[c])
                elif s == 4:
                    nc.sync.dma_start(out=wt[1][:], in_=wtd[1][:])
                elif s == 5:
                    nc.sync.dma_start(out=wt[2][:], in_=wtd[2][:])
                # layer-0 z for node chunk s (nodes of sentence s)
                z_chunk(0, s)

            # ---------------- GAT layers ----------------
            for li in range(3):
                hout = hts[(li + 1) % 2]
                elrow_prep()
                P = {}
                esq = {}
                ystash, estash, rstash = {}, {}, {}

                def es_lm(half, c):
                    es = ps.tile([128, 512], f32, name="ps", tag="ps")
                    nc.tensor.matmul(es[:], identf[:],
                                     lm[:, c, half * 512:half * 512 + 512],
                                     start=True, stop=False)
                    esq[(half, c)] = es

                def es_fin(half, c):
                    es = esq[(half, c)]
                    nc.tensor.matmul(es[:], oneer2[:, c * 128:(c + 1) * 128],
                                     oneer[:, half * 512:half * 512 + 512],
                                     start=False, stop=True)
                    a = tpool.tile([128, 512], bf16, name="a", tag="a")
                    nc.scalar.activation(a[:], es[:], AF.Exp,
                                         bias=elpa[:, c:c + 1])
                    b = tpool.tile([128, 512], bf16, name="b", tag="b")
                    nc.scalar.activation(b[:], es[:], AF.Exp,
                                         bias=elpb[:, c:c + 1], scale=NEG)
                    if c % 2 == 0:
                        P[(half, c // 2)] = ppool.tile(
                            [128, 2, 512], f8e5, name="P", tag="P")
                    nc.vector.tensor_max(P[(half, c // 2)][:, c % 2, :],
                                         a[:], b[:])

                def csum(half, cp, csp):
                    nc.tensor.matmul(csp[:], ones_dr[:], P[(half, cp)][:],
                                     start=(cp == 0), stop=(cp == NSC - 1),
                                     perf_mode=DR)

                def recip(csp):
                    rb = rbp.tile([128, 512], f32, name="rb", tag="rb")
                    nc.vector.reciprocal_approx_fast(rb[:], csp[:])
                    return rb

                def agg_mm(half, cp, aggp):
                    for mi in range(6):
                        nc.tensor.matmul(
                            aggp[mi][:],
                            zt[:, 2 * cp:2 * cp + 2, mi * 128:(mi + 1) * 128],
                            P[(half, cp)][:],
                            start=(cp == 0), stop=(cp == NSC - 1),
                            perf_mode=DR)

                def yelu_mi(half, rb, aggt, mi, stage=None):
                    # three batched passes: ymuls, then exp/relu, then mins -
                    # keeps each engine queue free of cross-engine waits
                    if stage in (None, 0):
                        y = ypool.tile([128, 512], bf16, name="y",
                                       tag=f"y{mi}")
                        ystash[(half, mi)] = y
                        nc.vector.tensor_mul(y[:], aggt[:], rb[:])
                    if stage in (None, 1):
                        y = ystash[(half, mi)]
                        e = epool.tile([128, 512], bf16, name="e",
                                       tag=f"e{mi}")
                        estash[(half, mi)] = e
                        nc.scalar.activation(e[:], y[:], AF.Exp)
                        r = rpool.tile([128, 512], bf16, name="r",
                                       tag=f"r{mi}")
                        rstash[(half, mi)] = r
                        if li == 2 or mi % 2 == 0:
                            nc.scalar.activation(r[:], y[:], AF.Relu)
                        else:
                            nc.vector.tensor_scalar_max(r[:], y[:], 0.0)
                    if stage in (None, 2):
                        e, r = estash[(half, mi)], rstash[(half, mi)]
                        # elu = min(exp(y)-1, relu(y))
                        if li == 2:
                            dst = h3s[mi % 2][:]
                            nc.vector.scalar_tensor_tensor(
                                dst, e[:], 1.0, r[:], OP.subtract, OP.min,
                                accum_out=avacc[:, half * 6 + mi:
                                                half * 6 + mi + 1])
                        else:
                            dst = hout[:, mi // 2, mi % 2,
                                       half * 512:half * 512 + 512]
                            nc.vector.scalar_tensor_tensor(
                                dst, e[:], 1.0, r[:], OP.subtract, OP.min)

                def agg4(half, aggt, mi):
                    for cp in range(NSC):
                        nc.tensor.matmul(
                            aggt[:],
                            zt[:, 2 * cp:2 * cp + 2, mi * 128:(mi + 1) * 128],
                            P[(half, cp)][:],
                            start=(cp == 0), stop=(cp == NSC - 1),
                            perf_mode=DR)

                # half-0 P production (LM adds first to cover el/er DMA chain)
                for c in range(NCH):
                    es_lm(0, c)
                for c in range(NCH):
                    es_fin(0, c)
                csp0 = ps.tile([128, 512], f32, name="ps", tag="ps")
                for cp in range(NSC):
                    csum(0, cp, csp0)
                rb0 = recip(csp0)
                # agg half-0 (cp-outer: banks rotate, PE pipelines)
                # overlapped with half-1 P production (es double-buffered)
                aggp0 = [ps.tile([128, 512], f32, name="ps", tag="ps")
                         for _ in range(6)]
                for cp in range(NSC):
                    for mi in range(6):
                        nc.tensor.matmul(
                            aggp0[mi][:],
                            zt[:, 2 * cp:2 * cp + 2, mi * 128:(mi + 1) * 128],
                            P[(0, cp)][:],
                            start=(cp == 0), stop=(cp == NSC - 1),
                            perf_mode=DR)
                    es_lm(1, 2 * cp)
                    es_fin(1, 2 * cp)
                    es_lm(1, 2 * cp + 1)
                    es_fin(1, 2 * cp + 1)
                csp1 = ps.tile([128, 512], f32, name="ps", tag="ps")
                for cp in range(NSC):
                    csum(1, cp, csp1)
                rb1 = recip(csp1)
                for st in range(3):
                    for mi in range(6):
                        yelu_mi(0, rb0, aggp0[mi], mi, stage=st)
                # agg half-1 overlapped with next layer's first z chunks
                aggp1 = [ps.tile([128, 512], f32, name="ps", tag="ps")
                         for _ in range(6)]
                for cp in range(NSC):
                    for mi in range(6):
                        nc.tensor.matmul(
                            aggp1[mi][:],
                            zt[:, 2 * cp:2 * cp + 2, mi * 128:(mi + 1) * 128],
                            P[(1, cp)][:],
                            start=(cp == 0), stop=(cp == NSC - 1),
                            perf_mode=DR)
                    if li < 2 and cp >= 1:
                        z_chunk(li + 1, cp - 1)
                if li < 2:
                    z_chunk(li + 1, 3)
                for st in range(3):
                    for mi in range(6):
                        yelu_mi(1, rb1, aggp1[mi], mi, stage=st)
                if li < 2:
                    for c in range(4, NCH):
                        z_chunk(li + 1, c)

            # ---------------- final reduction (batched) ----------------
            qfT6 = rpool.tile([128, 6], f32, name="qfT6", tag="qfT6")
            nc.vector.tensor_reduce(qfT6[:], qfacc[:], AX.X, OP.add)
            u6 = rpool.tile([128, 6], f32, name="u6", tag="u6")
            nc.vector.tensor_add(u6[:], avacc[:, 0:6], avacc[:, 6:12])
            nc.vector.scalar_tensor_tensor(
                dfin[:], u6[:], 1.0 / NPD, qfT6[:], OP.mult, OP.subtract)
            dfa = rpool.tile([128, 6], f32, name="dfa", tag="dfa")
            nc.scalar.activation(dfa[:], dfin[:], AF.Abs)
            dfr = rpool.tile([128, 1], f32, name="dfr", tag="dfr")
            nc.vector.tensor_reduce(dfr[:], dfa[:], AX.X, OP.add)
            from concourse import bass_isa
            finp = rpool.tile([128, 1], f32, name="finp", tag="finp")
            nc.gpsimd.partition_all_reduce(finp[:], dfr[:], 128,
                                           bass_isa.ReduceOp.add)
            nc.sync.dma_start(out=out_d[:], in_=finp[0:1, :])

    nc.finalize()
    return nc


def _shard_inputs(inputs):
    """Host-side preprocessing: build per-core input maps."""
    import ml_dtypes
    f8 = ml_dtypes.float8_e4m3
    perm = _perm()

    f = np.asarray(inputs["features"], np.float32)
    spans = np.asarray(inputs["token_spans"])
    masks = np.asarray(inputs["masks"])
    sel = np.asarray(inputs["selected_indices"])
    src = np.asarray(inputs["src"])
    dst = np.asarray(inputs["dst"])
    doc_spans = np.asarray(inputs["doc_spans"])
    seg = np.asarray(inputs["segment_ids"])
    ish = np.asarray(inputs["is_head"])
    B = f.shape[0]

    pos = np.arange(L)
    bi = np.arange(B)[:, None]
    stx = spans[bi, sel, 0]
    en = spans[bi, sel, 1]
    sm = ((pos[None, None, :] >= stx[:, :, None])
          & (pos[None, None, :] < en[:, :, None])
          & (masks[:, None, :] > 0)).astype(np.float32)
    cnt = np.maximum(sm.sum(-1), 1.0)
    w = sm / cnt[:, :, None] * (en > 0).astype(np.float32)[:, :, None]  # [B,SEL,L]
    doc_cnt = np.maximum((doc_spans[:, 1] - doc_spans[:, 0]).astype(np.float32), 1.0)
    qm = (((ish != 2) & (seg == 0) & (masks > 0)).astype(np.float32))   # [B,L]
    smask_all = np.zeros((B, L, SW), np.float32)
    smask_all[:, :, :SEL] = w.transpose(0, 2, 1)

    # shared weight tensors
    wts = []
    for i in range(3):
        W = np.asarray(inputs[f"W{i}"], np.float32)
        al = np.asarray(inputs[f"al{i}"], np.float32)
        ar = np.asarray(inputs[f"ar{i}"], np.float32)
        Wp = W[perm][:, perm]
        wal = (W @ al)[perm]
        war = (W @ ar)[perm]
        Wf = np.zeros((H, WC), np.float32)
        Wf[:, :H] = Wp
        Wf[:, H] = wal
        Wf[:, H + 1] = war
        wts.append(Wf.reshape(KH2, 2, 128, WC).transpose(2, 0, 1, 3).astype(f8))
    identb = np.eye(128, dtype=np.float32)
    identf = np.eye(128, dtype=f8)

    in_maps = []
    for d in range(D):
        s0, s1 = int(doc_spans[d, 0]), int(doc_spans[d, 1])
        assert s1 - s0 == SPD, "kernel assumes 8 sentences per doc"
        sm_d = smask_all[s0:s1].copy()
        sm_d[:, :, SEL] = qm[s0:s1] / doc_cnt[d]
        f_d = f[s0:s1][:, :, perm]                      # [SPD, L, H] permuted
        lo, hi = d * NPD, (d + 1) * NPD
        eidx = np.where((dst >= lo) & (dst < hi))[0]
        ls = src[eidx] - lo
        ld = dst[eidx] - lo
        assert np.all((ls >= 0) & (ls < NPD)), "edge crosses doc block"
        M = np.bincount(ls * NPD + ld, minlength=NPD * NPD).astype(np.float32)
        M = M.reshape(NPD, NPD)
        LM = np.where(M > 0, np.log(np.maximum(M, 1.0)), -200.0).astype(np.float32)
        fs = np.concatenate([f_d, sm_d], axis=2)     # [SPD, L, H+SW]
        im = {
            # L interleave: l = 256*k2 + 2p + i
            "fs": fs.reshape(SPD, 2, 128, 2, H + SW).transpose(0, 2, 1, 3, 4)
                    .astype(f8),
            "lm": LM.reshape(NCH, 128, NPD).astype(f8),
            "idb": identb, "idf": identf,
            "zo": np.concatenate([np.zeros((1, NPD), np.float32),
                                  np.ones((1, NPD), np.float32)]
                                 ).astype(ml_dtypes.bfloat16),
        }
        for i in range(3):
            im[f"W{i}"] = wts[i]
        in_maps.append(im)
    return in_maps


def _run(inputs, trace=False, tmpdir=None):
    _ensure_env()
    from concourse.bass_utils import run_bass_kernel_spmd
    if "nc" not in _PROG:
        _PROG["nc"] = _build_program()
    in_maps = _shard_inputs(inputs)
    res = run_bass_kernel_spmd(_PROG["nc"], in_maps, core_ids=list(range(D)),
                               trace=trace, tmpdir=tmpdir)
    out = np.array([res.results[c]["out"][0, 0] for c in range(D)], np.float32)
    return out, res


def kernel(**inputs) -> np.ndarray:
    out, _ = _run(inputs)
    return out
